# revision 8
# baseline (speedup 1.0000x reference)
"""DeepseekV3 attention on 8 TRN2 NeuronCores.

v2: phase 1 token-sharded latent projections (8 blocks of 512 tokens),
AllGather of kv then q latents within each 4-core batch group, phase 2
head-sharded (4 heads per core) attention + partial o_proj in fp16;
host sums the 4 partials per batch.

vs v1: single merged q AllGather; causal mask applied as a -3000 bias
matmul into the score PSUM (keeps DVE off the PV critical path); exp in
[128,1024] groups; exp-sum accumulation on gpsimd; o_proj interleaved
per query block; PSUM->SBUF copies split between vector and scalar
engines; PE keep-warm chains through the AllGather windows; fp16
partial output.
"""
import numpy as np
import ml_dtypes

import concourse.bacc as bacc
import concourse.mybir as mybir
import concourse.tile as tile

B, T, HID = 2, 2048, 2048
NH = 16
QLR, KVLR = 1536, 512
DN, DR = 128, 64
DQK, DV = DN + DR, 128
EPS = 1e-6
THETA = 10000.0
SCALE = DQK ** -0.5

NB = 512          # tokens per phase-1 block
HPC = 4           # heads per core in phase 2
KVR = KVLR + 2 * DR  # 640 rows: kv_lat | krot | krotswap

f32 = mybir.dt.float32
f16 = mybir.dt.float16
bf16 = mybir.dt.bfloat16
Exp = mybir.ActivationFunctionType.Exp
Sqrt = mybir.ActivationFunctionType.Sqrt
Square = mybir.ActivationFunctionType.Square

_BF = ml_dtypes.bfloat16


def _build():
    nc = bacc.Bacc(None, num_devices=8)

    # ---- per-core inputs ----
    xT = nc.declare_dram_parameter("xT", [HID, NB], bf16, isOutput=False)
    wqa = nc.declare_dram_parameter("wqa", [HID, QLR], bf16, isOutput=False)
    wkva = nc.declare_dram_parameter("wkva", [HID, KVR], bf16, isOutput=False)
    wqb = nc.declare_dram_parameter("wqb", [QLR, 768], bf16, isOutput=False)
    sel = nc.declare_dram_parameter("sel", [2, 128, 128], bf16, isOutput=False)
    wkvk = nc.declare_dram_parameter("wkvk", [KVLR, 512], bf16, isOutput=False)
    wkvv = nc.declare_dram_parameter("wkvv", [KVLR, 512], bf16, isOutput=False)
    wo = nc.declare_dram_parameter("wo", [HPC * DV, HID], bf16, isOutput=False)
    cs = nc.declare_dram_parameter("cs", [128, T], bf16, isOutput=False)  # [c;c;-s;s]
    umask = nc.declare_dram_parameter("umask", [4, 128, 512], bf16, isOutput=False)
    eye2 = nc.declare_dram_parameter("eye2", [128, 64], bf16, isOutput=False)
    eyeq = nc.declare_dram_parameter("eyeq", [128, 128], bf16, isOutput=False)
    out = nc.declare_dram_parameter("out", [T, HID], f16, isOutput=True)

    ag_in_kv = nc.dram_tensor("ag_in_kv", [KVR, NB], bf16)
    ag_out_kv = nc.dram_tensor("ag_out_kv", [4, KVR, NB], bf16)
    ag_in_q = nc.dram_tensor("ag_in_q", [QLR, NB], bf16)
    ag_out_q = nc.dram_tensor("ag_out_q", [4, QLR, NB], bf16)
    wsink = nc.dram_tensor("wsink", [128, 128], f32)

    RG = [[0, 1, 2, 3], [4, 5, 6, 7]]

    with tile.TileContext(nc) as tc:
        with tc.tile_pool(name="pcw", bufs=1) as pcw:    # tiny persistent consts
            onesb = pcw.tile([128, 128], bf16, tag="onesb")
            nc.vector.memset(onesb[:], 1.0)

            # ============ phase 1: latents for own 512-token block ============
            with (
                tc.tile_pool(name="p1", bufs=1) as p1,
                tc.tile_pool(name="p1w", bufs=2) as p1w,
                tc.tile_pool(name="ps1", bufs=3, space="PSUM") as ps1,
                tc.tile_pool(name="ps1acc", bufs=1, space="PSUM") as ps1acc,
            ):
                xt = p1.tile([128, 16, NB], bf16, tag="xt")
                nc.sync.dma_start(xt[:], xT[:, :].rearrange("(k p) t -> p k t", p=128))
                wkvat = p1.tile([128, 16, KVR], bf16, tag="wkvat")
                nc.sync.dma_start(
                    wkvat[:], wkva[:, :].rearrange("(k p) n -> p k n", p=128))
                wqat = p1.tile([128, 16, QLR], bf16, tag="wqat")
                nc.sync.dma_start(
                    wqat[:], wqa[:, :].rearrange("(k p) n -> p k n", p=128))

                def rmsnorm_store(ssq, latt, mt, d, dst_):
                    mt_ = p1w.tile([128, NB], f32, tag="rmst", name="rmst")
                    nc.vector.tensor_scalar(
                        mt_[:], ssq[:], 1.0 / d, EPS,
                        mybir.AluOpType.mult, mybir.AluOpType.add,
                    )
                    rms = p1w.tile([128, NB], f32, tag="rms", name="rms")
                    nc.scalar.activation(rms[:], mt_[:], Sqrt)
                    inv = p1w.tile([128, NB], f32, tag="inv", name="inv")
                    nc.vector.reciprocal(inv[:], rms[:])
                    for m in range(mt):
                        ltn = p1w.tile([128, NB], bf16, tag="ltn", name="ltn")
                        nc.vector.tensor_mul(ltn[:], latt[:, m], inv[:])
                        nc.gpsimd.dma_start(dst_[128 * m: 128 * (m + 1), :], ltn[:])

                # ckv^T first: m 0..3 kv_lat (normed), m 4 = krot+krotswap (raw)
                kvlat = p1.tile([128, 4, NB], f32, tag="kvlat")
                ssq_kv = ps1acc.tile([128, NB], f32, tag="ssq_kv")
                for m in range(5):
                    ps = ps1.tile([128, NB], f32, tag="p1ps")
                    for k in range(16):
                        nc.tensor.matmul(
                            ps[:], wkvat[:, k, 128 * m: 128 * (m + 1)], xt[:, k],
                            start=(k == 0), stop=(k == 15),
                        )
                    if m < 4:
                        sq = p1w.tile([128, NB], bf16, tag="sq")
                        nc.scalar.activation(sq[:], ps[:], Square)
                        nc.vector.tensor_copy(kvlat[:, m], ps[:])
                        nc.tensor.matmul(ssq_kv[:], onesb[:], sq[:],
                                         start=(m == 0), stop=(m == 3))
                    else:
                        rot = p1w.tile([128, NB], bf16, tag="rot")
                        nc.vector.tensor_copy(rot[:], ps[:])
                        nc.gpsimd.dma_start(ag_in_kv[KVLR:KVR, :], rot[:])
                rmsnorm_store(ssq_kv, kvlat, 4, KVLR, ag_in_kv)

                nc.gpsimd.collective_compute(
                    "AllGather", mybir.AluOpType.bypass,
                    replica_groups=RG, ins=[ag_in_kv[:]], outs=[ag_out_kv[:]],
                )

                # q latents, single 512-wide block
                qlat = p1.tile([128, 12, NB], f32, tag="qlat")
                ssq_q = ps1acc.tile([128, NB], f32, tag="ssq_q")
                for m in range(12):
                    ps = ps1.tile([128, NB], f32, tag="p1psq", name="p1psq")
                    for k in range(16):
                        nc.tensor.matmul(
                            ps[:], wqat[:, k, 128 * m: 128 * (m + 1)], xt[:, k],
                            start=(k == 0), stop=(k == 15),
                        )
                    sq = p1w.tile([128, NB], bf16, tag="sqq", name="sqq")
                    nc.scalar.activation(sq[:], ps[:], Square)
                    nc.vector.tensor_copy(qlat[:, m], ps[:])
                    nc.tensor.matmul(ssq_q[:], onesb[:], sq[:],
                                     start=(m == 0), stop=(m == 11))
                rmsnorm_store(ssq_q, qlat, 12, QLR, ag_in_q)
                nc.gpsimd.collective_compute(
                    "AllGather", mybir.AluOpType.bypass,
                    replica_groups=RG, ins=[ag_in_q[:]], outs=[ag_out_q[:]],
                )

            # phase-2 weight loads + persistent tiles (pools open after
            # phase-1 pools close so SBUF footprints don't overlap)
            pc = tc.alloc_tile_pool(name="pc", bufs=1)
            pcw2 = tc.alloc_tile_pool(name="pcw2", bufs=1)
            wqbt = pcw2.tile([128, 12, 768], bf16, tag="wqbt")
            nc.sync.dma_start(wqbt[:], wqb[:, :].rearrange("(k p) n -> p k n", p=128))
            wkkt = pcw2.tile([128, 4, 512], bf16, tag="wkkt")
            nc.sync.dma_start(wkkt[:], wkvk[:, :].rearrange("(k p) n -> p k n", p=128))
            wkvt = pcw2.tile([128, 4, 512], bf16, tag="wkvt")
            nc.sync.dma_start(wkvt[:], wkvv[:, :].rearrange("(k p) n -> p k n", p=128))
            cst = pcw2.tile([128, T], bf16, tag="cst")
            nc.sync.dma_start(cst[:], cs[:, :])
            eyet = pcw2.tile([128, 64], bf16, tag="eyet")
            nc.sync.dma_start(eyet[:], eye2[:, :])
            eyeqt = pcw2.tile([128, 128], bf16, tag="eyeqt")
            nc.sync.dma_start(eyeqt[:], eyeq[:, :])
            selt = pcw2.tile([128, 2, 128], bf16, tag="selt")
            for v in range(2):
                nc.sync.dma_start(selt[:, v], sel[v])
            umt = pcw2.tile([128, 4, 512], bf16, tag="umt")
            for m in range(4):
                nc.sync.dma_start(umt[:, m], umask[m])

            def warm_chain(n_pairs, tag):
                # paced PE activity through a collective wait window
                with (
                    tc.tile_pool(name=f"wc{tag}", bufs=1) as wp,
                    tc.tile_pool(name=f"wcp{tag}", bufs=1, space="PSUM") as wps,
                ):
                    a = wp.tile([128, 4096], bf16, tag="wa", name="wa")
                    b = wp.tile([128, 4096], bf16, tag="wb", name="wb")
                    nc.vector.memset(a[:], 1.0)
                    w = wps.tile([128, 128], f32, tag="wps", name="wps")
                    for i in range(n_pairs):
                        dst, src = (b, a) if i % 2 == 0 else (a, b)
                        nc.vector.tensor_copy(dst[:], src[:])
                        nc.tensor.matmul(w[:], dst[:, :128], dst[:, 128:256],
                                         start=True, stop=True)
                    sk = wp.tile([128, 128], f32, tag="wsk", name="wsk")
                    nc.vector.tensor_copy(sk[:], w[:])
                    nc.gpsimd.dma_start(wsink[:, :], sk[:])

            warm_chain(8, "a")

            # persistent phase-2 tiles
            qTp = pc.tile([128, 4, 4, NB], bf16, tag="qTp")      # [dn, h, qn, t]
            qrw = pc.tile([128, 2, 4, NB], bf16, tag="qrw")      # raw rot pairs
            qrot = [
                pc.tile([64, T], bf16, tag=f"qrot{h}", name=f"qrot{h}")
                for h in range(HPC)
            ]
            krotT = pc.tile([64, T], bf16, tag="krotT")
            kpT = pc.tile([128, 4, 4, NB], bf16, tag="kpT")      # [dn, h, r, t]
            vT = pc.tile([128, 16, 512], bf16, tag="vT")         # [t, t-tile, dv]
            attnT = pc.tile([128, 4, 4, NB], bf16, tag="attnT")  # [dv, h, qn, t]

            # ---- 2b: k_pass^T, V, k_rot rope (needs ag_out_kv) ----
            with (
                tc.tile_pool(name="p2b", bufs=1) as p2b,
                tc.tile_pool(name="p2bw", bufs=2) as p2bw,
                tc.tile_pool(name="ps2", bufs=2, space="PSUM") as ps2,
            ):
                kvl = p2b.tile([128, 4, 4, NB], bf16, tag="kvl")  # [r_lat, k, r, t]
                krr = p2b.tile([128, 4, NB], bf16, tag="krr")
                for r in range(4):
                    nc.scalar.dma_start(
                        kvl[:, :, r],
                        ag_out_kv[r, 0:KVLR, :].rearrange("(k p) t -> p k t", p=128),
                    )
                    nc.scalar.dma_start(krr[:, r], ag_out_kv[r, KVLR:KVR, :])
                # k_pass^T
                for m in range(4):
                    for r in range(4):
                        ps = ps2.tile([128, NB], f32, tag="k2ps")
                        for k in range(4):
                            nc.tensor.matmul(
                                ps[:], wkkt[:, k, 128 * m: 128 * (m + 1)],
                                kvl[:, k, r], start=(k == 0), stop=(k == 3),
                            )
                        if (m + r) % 2 == 0:
                            nc.vector.tensor_copy(kpT[:, m, r], ps[:])
                        else:
                            nc.scalar.copy(kpT[:, m, r], ps[:])
                # V token-major
                for r in range(4):
                    for s in range(4):
                        ps = ps2.tile([128, 512], f32, tag="v2ps")
                        for k in range(4):
                            nc.tensor.matmul(
                                ps[:], kvl[:, k, r, 128 * s: 128 * (s + 1)],
                                wkvt[:, k], start=(k == 0), stop=(k == 3),
                            )
                        if (r + s) % 2 == 0:
                            nc.scalar.copy(vT[:, 4 * r + s], ps[:])
                        else:
                            nc.vector.tensor_copy(vT[:, 4 * r + s], ps[:])
                # k_rot rope
                for r in range(4):
                    tt = p2bw.tile([128, NB], bf16, tag="kropet")
                    nc.vector.tensor_mul(
                        tt[:], krr[:, r], cst[:, 512 * r: 512 * (r + 1)]
                    )
                    pr = ps2.tile([64, NB], f32, tag="kropeps")
                    nc.tensor.matmul(pr[:], eyet[:], tt[:], start=True, stop=True)
                    nc.vector.tensor_copy(krotT[:, 512 * r: 512 * (r + 1)], pr[:])

            # o_proj weights (deferred load to bound phase-1 SBUF peak)
            wot = pcw2.tile([128, 4, HID], bf16, tag="wot")
            nc.sync.dma_start(wot[:], wo[:, :].rearrange("(k p) n -> p k n", p=128))

            warm_chain(22, "b")

            # ---- 2a: q^T (6 m-tiles x 4 blocks, K=12) + rope ----
            with (
                tc.tile_pool(name="p2a", bufs=2) as p2a,
                tc.tile_pool(name="p2as", bufs=2) as p2as,
                tc.tile_pool(name="ps2a", bufs=2, space="PSUM") as ps2a,
            ):
                for r in range(4):
                    qlt = p2a.tile([128, 12, NB], bf16, tag="qlt")
                    nc.scalar.dma_start(
                        qlt[:],
                        ag_out_q[r, :, :].rearrange("(k p) t -> p k t", p=128),
                    )
                    for m in range(6):
                        ps = ps2a.tile([128, NB], f32, tag="q2ps")
                        for k in range(12):
                            nc.tensor.matmul(
                                ps[:], wqbt[:, k, 128 * m: 128 * (m + 1)],
                                qlt[:, k], start=(k == 0), stop=(k == 11),
                            )
                        dst = qTp[:, m, r] if m < 4 else qrw[:, m - 4, r]
                        if m % 2 == 0:
                            nc.vector.tensor_copy(dst, ps[:])
                        else:
                            nc.scalar.copy(dst, ps[:])
                # rope q
                for h in range(HPC):
                    for r in range(4):
                        sp = ps2a.tile([128, NB], f32, tag="selps", name="selps")
                        nc.tensor.matmul(sp[:], selt[:, h % 2],
                                         qrw[:, h // 2, r], start=True, stop=True)
                        tt = p2as.tile([128, NB], bf16, tag="ropet")
                        nc.vector.tensor_mul(
                            tt[:], sp[:], cst[:, 512 * r: 512 * (r + 1)]
                        )
                        pr = ps2a.tile([64, NB], f32, tag="ropeps", name="ropeps")
                        nc.tensor.matmul(pr[:], eyet[:], tt[:], start=True, stop=True)
                        if r % 2 == 0:
                            nc.vector.tensor_copy(
                                qrot[h][:, 512 * r: 512 * (r + 1)], pr[:])
                        else:
                            nc.scalar.copy(
                                qrot[h][:, 512 * r: 512 * (r + 1)], pr[:])

            # ---- 2d attention + 2e o_proj interleaved per query block ----
            with (
                tc.tile_pool(name="p2d", bufs=4) as p2d,
                tc.tile_pool(name="p2dn", bufs=2) as p2dn,
                tc.tile_pool(name="p2eo", bufs=3) as p2eo,
                tc.tile_pool(name="ps_sc", bufs=2, space="PSUM") as ps_sc,
                tc.tile_pool(name="ps_ap", bufs=2, space="PSUM") as ps_ap,
                tc.tile_pool(name="ps_sp", bufs=1, space="PSUM") as ps_sp,
                tc.tile_pool(name="ps_o", bufs=1, space="PSUM") as ps_o,
            ):
                for qn in range(4):
                    nkt = 4 * qn + 4
                    for h in range(HPC):
                        aps = ps_ap.tile([128, NB], f32, tag="attn_ps")
                        eacg = p2dn.tile([128, NB], bf16, tag="eacg", name="eacg")
                        for g in range(nkt // 2):
                            scp = ps_sc.tile([128, 2, NB], f32, tag="scps")
                            for s in range(2):
                                kt = 2 * g + s
                                r, sl = kt // 4, 128 * (kt % 4)
                                m = kt - 4 * qn
                                nc.tensor.matmul(
                                    scp[:, s], kpT[:, h, r, sl: sl + 128],
                                    qTp[:, h, qn], start=True, stop=False,
                                )
                                nc.tensor.matmul(
                                    scp[:, s],
                                    krotT[:, 128 * kt: 128 * kt + 128],
                                    qrot[h][:, 512 * qn: 512 * (qn + 1)],
                                    start=False, stop=(m < 0),
                                )
                                if m >= 0:
                                    nc.tensor.matmul(
                                        scp[:, s], eyeqt[:], umt[:, m],
                                        start=False, stop=True,
                                    )
                            et = p2d.tile([128, 2, NB], bf16, tag="expT")
                            nc.scalar.activation(et[:], scp[:], Exp, scale=SCALE)
                            for s in range(2):
                                kt = 2 * g + s
                                nc.tensor.matmul(
                                    aps[:], vT[:, kt, 128 * h: 128 * (h + 1)],
                                    et[:, s], start=(kt == 0), stop=(kt == nkt - 1),
                                )
                            if g == 0:
                                nc.gpsimd.tensor_add(eacg[:], et[:, 0], et[:, 1])
                            else:
                                nc.gpsimd.tensor_add(eacg[:], eacg[:], et[:, 0])
                                nc.gpsimd.tensor_add(eacg[:], eacg[:], et[:, 1])
                        sps = ps_sp.tile([128, NB], f32, tag="sum_ps")
                        nc.tensor.matmul(sps[:], onesb[:], eacg[:],
                                         start=True, stop=True)
                        rec = p2dn.tile([128, NB], f32, tag="rec", name="rec")
                        nc.vector.reciprocal(rec[:], sps[:])
                        nc.vector.tensor_mul(attnT[:, h, qn], aps[:], rec[:])
                    # ---- o_proj for this query block ----
                    for tt_ in range(4):
                        for n in range(4):
                            ps = ps_o.tile([128, 512], f32, tag="ops")
                            for k in range(4):
                                nc.tensor.matmul(
                                    ps[:], attnT[:, k, qn, 128 * tt_: 128 * (tt_ + 1)],
                                    wot[:, k, 512 * n: 512 * (n + 1)],
                                    start=(k == 0), stop=(k == 3),
                                )
                            ot = p2eo.tile([128, 512], f16, tag="oT")
                            if n % 2 == 0:
                                nc.scalar.copy(ot[:], ps[:])
                            else:
                                nc.vector.tensor_copy(ot[:], ps[:])
                            nc.sync.dma_start(
                                out[512 * qn + 128 * tt_: 512 * qn + 128 * (tt_ + 1),
                                    512 * n: 512 * (n + 1)], ot[:],
                            )

            pcw2.release()
            pc.release()

    nc.finalize()
    return nc


_NC = None


def _get_nc():
    global _NC
    if _NC is None:
        _NC = _build()
    return _NC


def _prep_inputs(x, attention_mask, positions, wqa, qa_scale, wqb, wkva,
                 kva_scale, wkvb, wo):
    x = np.asarray(x, np.float32)
    positions = np.asarray(positions)
    wqa = np.asarray(wqa, np.float32)
    wqb = np.asarray(wqb, np.float32) * np.asarray(qa_scale, np.float32)[:, None]
    wkva = np.asarray(wkva, np.float32)
    wkvb = np.asarray(wkvb, np.float32) * np.asarray(kva_scale, np.float32)[:, None]
    wo = np.asarray(wo, np.float32)

    # wkva augmented with swapped-rot columns
    kr = wkva[:, KVLR:]
    wkva_aug = np.concatenate(
        [wkva[:, :KVLR], kr, kr[:, DR // 2:], kr[:, : DR // 2]], axis=1
    ).astype(_BF)

    # umask[m][r, c] = 0 where c >= 128*m + r (allowed), else -3000
    rr = np.arange(128)[:, None]
    cc = np.arange(512)[None, :]
    umask = np.stack(
        [np.where(cc >= 128 * m + rr, 0.0, -3000.0) for m in range(4)]
    ).astype(_BF)

    eye2 = np.concatenate([np.eye(64), np.eye(64)], axis=0).astype(_BF)
    eyeq = np.eye(128).astype(_BF)

    # sel[v]: out rows [0:64]=src rows [64v:64v+64]; [64:128]=32-swapped copy
    sel = np.zeros((2, 128, 128), np.float32)
    for v in range(2):
        for i in range(64):
            sel[v, 64 * v + i, i] = 1.0
            sel[v, 64 * v + ((i + 32) % 64), 64 + i] = 1.0
    sel = sel.astype(_BF)

    # per-batch cos/sin stack [c; c; -s; s]
    inv_freq = 1.0 / (THETA ** (np.arange(0, DR, 2, dtype=np.float32) / DR))
    cs_b = []
    for b in range(B):
        ang = positions[b].astype(np.float32)[None, :] * inv_freq[:, None]
        c, s = np.cos(ang), np.sin(ang)
        cs_b.append(np.concatenate([c, c, -s, s], axis=0).astype(_BF))

    wqa_bf = wqa.astype(_BF)
    in_maps = []
    for core in range(8):
        b, j = core // 4, core % 4
        hs = [4 * (core % 4) + i for i in range(HPC)]
        # wqb per head-group: [pass x4 | rot per head x4]
        cols = [wqb[:, h * DQK: h * DQK + DN] for h in hs]
        for h in hs:
            cols.append(wqb[:, h * DQK + DN: (h + 1) * DQK])
        wqb_hg = np.concatenate(cols, axis=1).astype(_BF)
        wkvk_hg = np.concatenate(
            [wkvb[:, h * (DN + DV): h * (DN + DV) + DN] for h in hs], axis=1
        ).astype(_BF)
        wkvv_hg = np.concatenate(
            [wkvb[:, h * (DN + DV) + DN: (h + 1) * (DN + DV)] for h in hs], axis=1
        ).astype(_BF)
        wo_hg = wo[hs[0] * DV: (hs[-1] + 1) * DV, :].astype(_BF)
        xTb = np.ascontiguousarray(
            x[b, NB * j: NB * (j + 1), :].T).astype(_BF)
        in_maps.append({
            "xT": xTb,
            "wqa": wqa_bf,
            "wkva": wkva_aug,
            "wqb": wqb_hg,
            "wkvk": wkvk_hg,
            "wkvv": wkvv_hg,
            "wo": wo_hg,
            "cs": cs_b[b],
            "umask": umask,
            "eye2": eye2,
            "eyeq": eyeq,
            "sel": sel,
        })
    return in_maps


def _run(inputs, trace=False, trace_kwargs=None):
    from concourse.bass_utils import run_bass_kernel_spmd

    nc = _get_nc()
    in_maps = _prep_inputs(**inputs)
    res = run_bass_kernel_spmd(
        nc, in_maps, list(range(8)), trace=trace,
        trace_kwargs=trace_kwargs or {},
    )
    outs = np.zeros((B, T, HID), np.float32)
    for core in range(8):
        outs[core // 4] += res.results[core]["out"].astype(np.float32)
    return outs, res


def kernel(**inputs) -> np.ndarray:
    out, _ = _run(inputs)
    return out
